# revision 11
# baseline (speedup 1.0000x reference)
"""nn_GCNConv Trainium2 Bass kernel (8 NeuronCores, SPMD, no collectives).

Computation: out = segment_sum(features[src], dst, N) @ W + b
  features [10000,128] f32, edge_index [2,640000] i64, W [128,256], b [256]

Strategy (dense count-matrix SpMM -> pure streaming GEMM, no SWDGE gather):
  - The segment-sum is  agg = A @ F  where A[d,s] = #edges s->d.  The host
    builds A as a dense fp8 count matrix (counts are tiny ints, exact in
    fp8e4) sharded by dst: core c owns dst nodes [1280c, 1280c+1280).
  - Per core the device computes, over 80 src chunks of 128:
      aggT[f,d] += F_chunk[s,f]^T @ A_chunk[s,d]    (PE, bf16 x fp8, PSUM f32)
    with dst split into 3 PSUM groups (512/512/256 cols).  A streams from
    HBM in 4 blocks per group (double-buffered, issued alternately on the
    two HWDGE rings SP/Activation); F streams once and stays resident.
  - Stage 2 per 128-dst window:  out = aggT^T @ W + b  (bf16 matmul + DVE
    bias add); its PE work is delayed one group so the PE never waits on
    the PSUM->SBUF copy.  Output is written p-major ([128,10,256]) so each
    group's store is one big-line DMA; the host untransposes.
  - Host concatenates the 8 per-core outputs and truncates to 10000 rows.
    Only index bookkeeping (histogram/pack) happens on host; all arithmetic
    on feature values runs on device.
"""

import sys

import ml_dtypes
import numpy as np

_TRN_REPO = "/opt/trn_rl_repo"
if _TRN_REPO not in sys.path:
    sys.path.insert(0, _TRN_REPO)

import concourse.bass as bass  # noqa: E402
import concourse.mybir as mybir  # noqa: E402
import concourse.tile as tile  # noqa: E402
from concourse import bacc, bass_utils  # noqa: E402

# ---------------------------------------------------------------------------
# Workaround: this walrus build rejects >1 sync-wait on a CTRL instruction
# ("Too many sync wait commands"). Tile's tail drain attaches a wait for every
# live sem lane to one InstDrain; chunk them across single-wait nops instead.
import re as _re  # noqa: E402

import bass_rust as _bass_rust  # noqa: E402


def _clock_ticks(vc):
    m = _re.search(r"\[([0-9, ]*)\]", repr(vc))
    return [int(x) for x in m.group(1).split(",")] if m.group(1).strip() else []


def _drain_and_barrier(self, tick_clock, wait_clock):
    ticks = _clock_ticks(tick_clock.global_clock)
    nz = [(i, t) for i, t in enumerate(ticks) if t > 0]
    for i, t in nz:
        vc = _bass_rust.VectorClock()
        vc.require_at_least(i, t)
        nop = self.nc.sync.nop(nofuse=True, hint="tail_wait")
        wait_clock.add_sem_waits(nop.ins, tile.ScopedClock({None: vc}))
    self.nc.sync.drain()  # waits already carried by the nops (SP FIFO order)
    self.nc.all_engine_barrier()
    assert self.sems is not None
    popped = self.nc._tile_sem_poison_stack.pop()
    assert popped is self._sem_poison
    self.nc.clear_and_free_semaphores(list(self.sems.allocated().values()))
    self.nc.all_engine_barrier()


tile.TileContext._drain_and_barrier = _drain_and_barrier
# ---------------------------------------------------------------------------

P = 128            # SBUF partitions = window node count = src chunk size
C_IN = 128
C_OUT = 256
N_NODES = 10000
N_CORES = 8
DPC = 1280         # dst nodes per core
NWIN = DPC // P    # 10 dst windows per core
NCH = 79           # src chunks (10112 padded src rows / 128)
GROUPS = (512, 512, 256)   # dst columns per PSUM accumulation group
ROUNDS = (4, 8, 17, 17, 17, 16)   # src chunks per streamed round (sums to NCH)


def _build_kernel():
    nc = bacc.Bacc("TRN2")
    dt = mybir.dt

    a_d = [
        nc.dram_tensor(f"a{gi}", [P, NCH, ng], dt.float8e4, kind="ExternalInput")
        for gi, ng in enumerate(GROUPS)
    ]
    f_d = nc.dram_tensor("f", [P, NCH, C_IN], dt.bfloat16, kind="ExternalInput")
    w_d = nc.dram_tensor("w", [C_IN, C_OUT], dt.bfloat16, kind="ExternalInput")
    bb_d = nc.dram_tensor("bb", [P, C_OUT], dt.float32, kind="ExternalInput")
    out_d = nc.dram_tensor("out", [P, NWIN, C_OUT], dt.float16, kind="ExternalOutput")

    with tile.TileContext(nc) as tc:
        with (
            tc.tile_pool(name="consts", bufs=1) as cpool,
            tc.tile_pool(name="a", bufs=3) as apool,
            tc.tile_pool(name="agg", bufs=1) as aggpool,
            tc.tile_pool(name="o", bufs=2) as opool,
            tc.tile_pool(name="psa", bufs=3, space="PSUM") as psa,
            tc.tile_pool(name="pso", bufs=2, space="PSUM") as pso,
        ):
            # HAM pre-warm: a few dummy matmuls on zeroed scratch keep the PE
            # busy during the DMA head so the real stream starts at 2.4 GHz.
            warm_w = cpool.tile([P, C_IN], dt.bfloat16)
            warm_x = cpool.tile([P, 256], dt.float8e4)
            warm_p = psa.tile([P, 256], dt.float32, tag="warm", bufs=1)
            nc.gpsimd.memset(warm_w[:], 0.0)
            nc.gpsimd.memset(warm_x[:], 0.0)
            # >=3.4us of sustained PE busy (16 x 256/1.2GHz) trips the HAM
            # throttle to full clock before the real stream begins
            for _ in range(16):
                nc.tensor.matmul(warm_p[:], lhsT=warm_w[:], rhs=warm_x[:],
                                 start=True, stop=True)

            w_s = cpool.tile([C_IN, C_OUT], dt.bfloat16)
            bb_s = cpool.tile([P, C_OUT], dt.float32)
            f_s = cpool.tile([P, NCH, C_IN], dt.bfloat16)
            nc.gpsimd.dma_start(out=w_s[:], in_=w_d[:])
            nc.gpsimd.dma_start(out=bb_s[:], in_=bb_d[:])

            # three persistent PSUM accumulation groups, one per dst slice
            aggps = [
                psa.tile([P, ng], dt.float32, tag="aggp", padded_shape=[P, 512],
                         name=f"aggp{gi}")
                for gi, ng in enumerate(GROUPS)
            ]

            # dedicated issue streams, ordered to match per-round consumption:
            # scalar ring carries F then a1; sync ring carries a0 then a2
            a_eng = [nc.sync, nc.scalar, nc.sync]
            kb0 = 0
            for ri, kbn in enumerate(ROUNDS):
                nc.scalar.dma_start(
                    out=f_s[:, kb0 : kb0 + kbn, :], in_=f_d[:, kb0 : kb0 + kbn, :]
                )
                for gi, ng in enumerate(GROUPS):
                    a_t = apool.tile([P, kbn, ng], dt.float8e4, tag=f"g{gi}",
                                     padded_shape=[P, max(ROUNDS), ng],
                                     name=f"a_t{gi}_{ri}")
                    a_eng[gi].dma_start(
                        out=a_t[:, :kbn, :], in_=a_d[gi][:, kb0 : kb0 + kbn, :]
                    )
                    for k in range(kbn):
                        kk = kb0 + k
                        nc.tensor.matmul(
                            aggps[gi][:],
                            lhsT=f_s[:, kk, :],
                            rhs=a_t[:, k, :],
                            start=(kk == 0),
                            stop=(kk == NCH - 1),
                        )
                kb0 += kbn

            # stage 2: copies drain on DVE while the PE finishes the last
            # round; s2 matmuls are emitted after all accumulation matmuls
            agg_ss = []
            for gi, ng in enumerate(GROUPS):
                agg_s = aggpool.tile([P, ng], dt.bfloat16, tag=f"agg{gi}",
                                     name=f"agg_s{gi}")
                nc.vector.tensor_copy(agg_s[:], aggps[gi][:])
                agg_ss.append(agg_s)

            wbase = 0
            for gi, ng in enumerate(GROUPS):
                nw = ng // P
                for wl in range(nw):
                    outp = pso.tile([P, C_OUT], dt.float32, tag="op",
                                    name=f"op{gi}_{wl}")
                    nc.tensor.matmul(
                        outp[:],
                        lhsT=agg_ss[gi][:, wl * P : (wl + 1) * P],
                        rhs=w_s[:],
                        start=True,
                        stop=True,
                    )
                    out_t = opool.tile([P, C_OUT], dt.float16, tag="o",
                                       name=f"out_{gi}_{wl}")
                    nc.vector.tensor_add(out_t[:], outp[:], bb_s[:])
                    (nc.scalar if (wbase + wl) % 2 else nc.sync).dma_start(
                        out=out_d[:, wbase + wl, :], in_=out_t[:]
                    )
                wbase += nw

    nc.compile()
    return nc


def _prep_inputs(features, edge_index, W, b):
    """Host-side sharding: dense per-core fp8 count matrices + packed F/W/b."""
    src = np.asarray(edge_index[0]).astype(np.int64)
    dst = np.asarray(edge_index[1]).astype(np.int64)

    # A[core, p, c, dloc] = #edges (src = c*128+p) -> (dst = core*1280+dloc)
    A = np.zeros((N_CORES, P, NCH, DPC), np.uint8)
    flat = ((dst // DPC * P + src % P) * NCH + src // P) * DPC + dst % DPC
    np.add.at(A.reshape(-1), flat, 1)
    amax = int(A.max())
    assert amax <= 16, f"edge multiplicity {amax} not exact in fp8"
    A8 = A.astype(ml_dtypes.float8_e4m3)

    bounds = np.cumsum((0,) + GROUPS)
    a_groups = [
        np.ascontiguousarray(A8[:, :, :, bounds[gi] : bounds[gi + 1]])
        for gi in range(len(GROUPS))
    ]

    f16 = np.zeros((NCH * P, C_IN), ml_dtypes.bfloat16)
    f16[:N_NODES] = np.asarray(features, np.float32).astype(ml_dtypes.bfloat16)
    f_host = np.ascontiguousarray(f16.reshape(NCH, P, C_IN).transpose(1, 0, 2))
    w_host = np.asarray(W, np.float32).astype(ml_dtypes.bfloat16)
    bb_host = np.tile(np.asarray(b, np.float32)[None, :], (P, 1))

    in_maps = []
    for ci in range(N_CORES):
        m = {f"a{gi}": a_groups[gi][ci] for gi in range(len(GROUPS))}
        m.update({"f": f_host, "w": w_host, "bb": bb_host})
        in_maps.append(m)
    return in_maps


_KERNEL_CACHE: dict = {}


def _get_kernel():
    if "k" not in _KERNEL_CACHE:
        _KERNEL_CACHE["k"] = _build_kernel()
    return _KERNEL_CACHE["k"]


def kernel(features, edge_index, W, b):
    features = np.asarray(features, dtype=np.float32)
    edge_index = np.asarray(edge_index)
    W = np.asarray(W, dtype=np.float32)
    b = np.asarray(b, dtype=np.float32)
    assert features.shape == (N_NODES, C_IN), features.shape
    assert W.shape == (C_IN, C_OUT) and b.shape == (C_OUT,)

    in_maps = _prep_inputs(features, edge_index, W, b)
    nc = _get_kernel()
    res = bass_utils.run_bass_kernel_spmd(nc, in_maps, core_ids=list(range(N_CORES)))
    # out is [128, 10, 256] f16 p-major per core -> [1280, 256] node-major
    out = np.concatenate(
        [
            np.asarray(res.results[c]["out"], np.float32)
            .transpose(1, 0, 2)
            .reshape(DPC, C_OUT)
            for c in range(N_CORES)
        ],
        axis=0,
    )
    return np.ascontiguousarray(out[:N_NODES])


# revision 18
# speedup vs baseline: 1.0714x; 1.0714x over previous
"""nn_GCNConv Trainium2 Bass kernel (8 NeuronCores, SPMD, no collectives).

Computation: out = segment_sum(features[src], dst, N) @ W + b
  features [10000,128] f32, edge_index [2,640000] i64, W [128,256], b [256]

Strategy (dense count-matrix SpMM -> pure streaming GEMM, no SWDGE gather):
  - The segment-sum is  agg = A @ F  where A[d,s] = #edges s->d.  The host
    builds A as a dense fp8 count matrix (counts are tiny ints, exact in
    fp8e4) sharded by dst: core c owns dst nodes [1280c, 1280c+1280).
  - Per core the device computes, over 80 src chunks of 128:
      aggT[f,d] += F_chunk[s,f]^T @ A_chunk[s,d]    (PE, bf16 x fp8, PSUM f32)
    with dst split into 3 PSUM groups (512/512/256 cols).  A streams from
    HBM in 4 blocks per group (double-buffered, issued alternately on the
    two HWDGE rings SP/Activation); F streams once and stays resident.
  - Stage 2 per 128-dst window:  out = aggT^T @ W + b  (bf16 matmul + DVE
    bias add); its PE work is delayed one group so the PE never waits on
    the PSUM->SBUF copy.  Output is written p-major ([128,10,256]) so each
    group's store is one big-line DMA; the host untransposes.
  - Host concatenates the 8 per-core outputs and truncates to 10000 rows.
    Only index bookkeeping (histogram/pack) happens on host; all arithmetic
    on feature values runs on device.
"""

import sys

import ml_dtypes
import numpy as np

_TRN_REPO = "/opt/trn_rl_repo"
if _TRN_REPO not in sys.path:
    sys.path.insert(0, _TRN_REPO)

import concourse.bass as bass  # noqa: E402
import concourse.mybir as mybir  # noqa: E402
import concourse.tile as tile  # noqa: E402
from concourse import bacc, bass_utils  # noqa: E402

# ---------------------------------------------------------------------------
# Workaround: this walrus build rejects >1 sync-wait on a CTRL instruction
# ("Too many sync wait commands"). Tile's tail drain attaches a wait for every
# live sem lane to one InstDrain; chunk them across single-wait nops instead.
import re as _re  # noqa: E402

import bass_rust as _bass_rust  # noqa: E402


def _clock_ticks(vc):
    m = _re.search(r"\[([0-9, ]*)\]", repr(vc))
    return [int(x) for x in m.group(1).split(",")] if m.group(1).strip() else []


def _drain_and_barrier(self, tick_clock, wait_clock):
    ticks = _clock_ticks(tick_clock.global_clock)
    nz = [(i, t) for i, t in enumerate(ticks) if t > 0]
    for i, t in nz:
        vc = _bass_rust.VectorClock()
        vc.require_at_least(i, t)
        nop = self.nc.sync.nop(nofuse=True, hint="tail_wait")
        wait_clock.add_sem_waits(nop.ins, tile.ScopedClock({None: vc}))
    self.nc.sync.drain()  # waits already carried by the nops (SP FIFO order)
    self.nc.all_engine_barrier()
    assert self.sems is not None
    popped = self.nc._tile_sem_poison_stack.pop()
    assert popped is self._sem_poison
    self.nc.clear_and_free_semaphores(list(self.sems.allocated().values()))
    self.nc.all_engine_barrier()


tile.TileContext._drain_and_barrier = _drain_and_barrier
# ---------------------------------------------------------------------------

P = 128            # SBUF partitions = window node count = src chunk size
C_IN = 128
C_OUT = 256
N_NODES = 10000
N_CORES = 8
DPC = 1280         # dst nodes per core
NWIN = DPC // P    # 10 dst windows per core
NCH = 79           # src chunks (10112 padded src rows / 128)
GROUPS = (512, 512, 256)   # dst columns per PSUM accumulation group
ROUNDS = (4, 8, 17, 17, 17, 16)   # src chunks per streamed round (sums to NCH)
HDP = DPC // 2     # packed-A columns: two 4-bit counts per byte
# dst permutation: lo nibbles cover [0:512)+[1024:1152), hi the rest, so each
# PSUM group's matmul reads one contiguous slice of one unpacked tile
LO_DST = np.r_[0:512, 1024:1152]
HI_DST = np.r_[512:1024, 1152:1280]


def _build_kernel():
    nc = bacc.Bacc("TRN2")
    dt = mybir.dt

    pk_d = nc.dram_tensor("pk", [P, NCH, HDP], dt.uint8, kind="ExternalInput")
    f_d = nc.dram_tensor("f", [P, NCH, C_IN], dt.bfloat16, kind="ExternalInput")
    w_d = nc.dram_tensor("w", [C_IN, C_OUT], dt.bfloat16, kind="ExternalInput")
    bb_d = nc.dram_tensor("bb", [P, C_OUT], dt.float32, kind="ExternalInput")
    out_d = nc.dram_tensor("out", [P, NWIN, C_OUT], dt.float16, kind="ExternalOutput")

    with tile.TileContext(nc) as tc:
        with (
            tc.tile_pool(name="consts", bufs=1) as cpool,
            tc.tile_pool(name="a", bufs=3) as apool,
            tc.tile_pool(name="agg", bufs=1) as aggpool,
            tc.tile_pool(name="o", bufs=3) as opool,
            tc.tile_pool(name="psa", bufs=3, space="PSUM") as psa,
            tc.tile_pool(name="pso", bufs=2, space="PSUM") as pso,
        ):
            # HAM pre-warm: a few dummy matmuls on zeroed scratch keep the PE
            # busy during the DMA head so the real stream starts at 2.4 GHz.
            warm_w = cpool.tile([P, C_IN], dt.bfloat16)
            warm_x = cpool.tile([P, 256], dt.float8e4)
            warm_p = psa.tile([P, 256], dt.float32, tag="warm", bufs=1)
            nc.gpsimd.memset(warm_w[:], 0.0)
            nc.gpsimd.memset(warm_x[:], 0.0)
            # >=3.4us of sustained PE busy (16 x 256/1.2GHz) trips the HAM
            # throttle to full clock before the real stream begins
            for _ in range(16):
                nc.tensor.matmul(warm_p[:], lhsT=warm_w[:], rhs=warm_x[:],
                                 start=True, stop=True)

            w_s = cpool.tile([C_IN, C_OUT], dt.bfloat16)
            bb_s = cpool.tile([P, C_OUT], dt.float32)
            f_s = cpool.tile([P, NCH, C_IN], dt.bfloat16)
            nc.gpsimd.dma_start(out=w_s[:], in_=w_d[:])
            nc.gpsimd.dma_start(out=bb_s[:], in_=bb_d[:])

            # three persistent PSUM accumulation groups, one per dst slice
            aggps = [
                psa.tile([P, ng], dt.float32, tag="aggp", padded_shape=[P, 512],
                         name=f"aggp{gi}")
                for gi, ng in enumerate(GROUPS)
            ]

            # packed A streams at 4 bits/count; DVE unpacks nibbles into two
            # dense fp8 tiles (lo/hi) whose column slices feed the PSUM groups
            kb0 = 0
            for ri, kbn in enumerate(ROUNDS):
                nc.scalar.dma_start(
                    out=f_s[:, kb0 : kb0 + kbn, :], in_=f_d[:, kb0 : kb0 + kbn, :]
                )
                pk_t = apool.tile([P, kbn, HDP], dt.uint8, tag="pk",
                                  padded_shape=[P, max(ROUNDS), HDP],
                                  name=f"pk_t{ri}", bufs=2)
                nc.sync.dma_start(
                    out=pk_t[:, :kbn, :], in_=pk_d[:, kb0 : kb0 + kbn, :]
                )
                alo = apool.tile([P, kbn, HDP], dt.float8e4, tag="alo",
                                 padded_shape=[P, max(ROUNDS), HDP],
                                 name=f"alo{ri}")
                ahi = apool.tile([P, kbn, HDP], dt.float8e4, tag="ahi",
                                 padded_shape=[P, max(ROUNDS), HDP],
                                 name=f"ahi{ri}")
                nc.vector.tensor_scalar(
                    out=alo[:, :kbn, :], in0=pk_t[:, :kbn, :], scalar1=15,
                    scalar2=None, op0=mybir.AluOpType.bitwise_and,
                )
                nc.vector.tensor_scalar(
                    out=ahi[:, :kbn, :], in0=pk_t[:, :kbn, :], scalar1=4,
                    scalar2=None, op0=mybir.AluOpType.logical_shift_right,
                )
                for k in range(kbn):
                    kk = kb0 + k
                    st, sp = (kk == 0), (kk == NCH - 1)
                    nc.tensor.matmul(aggps[0][:], lhsT=f_s[:, kk, :],
                                     rhs=alo[:, k, 0:512], start=st, stop=sp)
                    nc.tensor.matmul(aggps[1][:], lhsT=f_s[:, kk, :],
                                     rhs=ahi[:, k, 0:512], start=st, stop=sp)
                    nc.tensor.matmul(aggps[2][:, 0:P], lhsT=f_s[:, kk, :],
                                     rhs=alo[:, k, 512:HDP], start=st, stop=sp)
                    nc.tensor.matmul(aggps[2][:, P : 2 * P], lhsT=f_s[:, kk, :],
                                     rhs=ahi[:, k, 512:HDP], start=st, stop=sp)
                kb0 += kbn

            # stage 2: copies drain on DVE while the PE finishes the last
            # round; s2 matmuls are emitted after all accumulation matmuls
            agg_ss = []
            for gi, ng in enumerate(GROUPS):
                agg_s = aggpool.tile([P, ng], dt.bfloat16, tag=f"agg{gi}",
                                     name=f"agg_s{gi}")
                nc.vector.tensor_copy(agg_s[:], aggps[gi][:])
                agg_ss.append(agg_s)

            wbase = 0
            for gi, ng in enumerate(GROUPS):
                nw = ng // P
                out_t = opool.tile([P, nw, C_OUT], dt.float16, tag="o",
                                   name=f"out_g{gi}")
                for wl in range(nw):
                    outp = pso.tile([P, C_OUT], dt.float32, tag="op",
                                    name=f"op{gi}_{wl}")
                    nc.tensor.matmul(
                        outp[:],
                        lhsT=agg_ss[gi][:, wl * P : (wl + 1) * P],
                        rhs=w_s[:],
                        start=True,
                        stop=True,
                    )
                    nc.vector.tensor_add(out_t[:, wl, :], outp[:], bb_s[:])
                (nc.scalar if gi % 2 else nc.sync).dma_start(
                    out=out_d[:, wbase : wbase + nw, :], in_=out_t[:]
                )
                wbase += nw

    nc.compile()
    return nc


def _prep_inputs(features, edge_index, W, b):
    """Host-side sharding: dense per-core fp8 count matrices + packed F/W/b."""
    src = np.asarray(edge_index[0]).astype(np.int64)
    dst = np.asarray(edge_index[1]).astype(np.int64)

    # A[core, p, c, dloc] = #edges (src = c*128+p) -> (dst = core*1280+dloc)
    A = np.zeros((N_CORES, P, NCH, DPC), np.uint8)
    flat = ((dst // DPC * P + src % P) * NCH + src // P) * DPC + dst % DPC
    np.add.at(A.reshape(-1), flat, 1)
    amax = int(A.max())
    assert amax <= 15, f"edge multiplicity {amax} does not fit a nibble"
    pk = A[:, :, :, LO_DST] | (A[:, :, :, HI_DST] << 4)
    pk = np.ascontiguousarray(pk)

    f16 = np.zeros((NCH * P, C_IN), ml_dtypes.bfloat16)
    f16[:N_NODES] = np.asarray(features, np.float32).astype(ml_dtypes.bfloat16)
    f_host = np.ascontiguousarray(f16.reshape(NCH, P, C_IN).transpose(1, 0, 2))
    w_host = np.asarray(W, np.float32).astype(ml_dtypes.bfloat16)
    bb_host = np.tile(np.asarray(b, np.float32)[None, :], (P, 1))

    in_maps = [
        {"pk": pk[ci], "f": f_host, "w": w_host, "bb": bb_host}
        for ci in range(N_CORES)
    ]
    return in_maps


_KERNEL_CACHE: dict = {}


def _get_kernel():
    if "k" not in _KERNEL_CACHE:
        _KERNEL_CACHE["k"] = _build_kernel()
    return _KERNEL_CACHE["k"]


def kernel(features, edge_index, W, b):
    features = np.asarray(features, dtype=np.float32)
    edge_index = np.asarray(edge_index)
    W = np.asarray(W, dtype=np.float32)
    b = np.asarray(b, dtype=np.float32)
    assert features.shape == (N_NODES, C_IN), features.shape
    assert W.shape == (C_IN, C_OUT) and b.shape == (C_OUT,)

    in_maps = _prep_inputs(features, edge_index, W, b)
    nc = _get_kernel()
    res = bass_utils.run_bass_kernel_spmd(nc, in_maps, core_ids=list(range(N_CORES)))
    # out is [128, 10, 256] f16 p-major per core -> [1280, 256] node-major
    out = np.concatenate(
        [
            np.asarray(res.results[c]["out"], np.float32)
            .transpose(1, 0, 2)
            .reshape(DPC, C_OUT)
            for c in range(N_CORES)
        ],
        axis=0,
    )
    return np.ascontiguousarray(out[:N_NODES])
